# revision 21
# baseline (speedup 1.0000x reference)
"""NSMCell (ins_id=0 branch) Trainium2 Bass kernel.

Full-input contract: kernel(**inputs) takes the unsharded numpy inputs and
returns the full (32, 512) softmax output. Internally shards batch B=32
across 8 NeuronCores (4 batches each); all compute per (b, n) is local to a
core, so no collectives are needed.

Math (per core, b local):
  z[p,n,k] = sum_h x[b,n,p,h] * instr[b,h] * sim[b,p] * W[p,h,k]
  A = sum_p z ;  Q = sum_p z^2
  s = A * exp(-0.5 * ln(Q + 1e-24))          # == A / max(sqrt(Q), 1e-12)
  e2 = max(s,0) + exp(min(s,0))              # == elu(s) + 1 (softmax-invariant)
  scores[n] = sum_k e2[n,k] * w_state[k]     # + const, softmax-invariant
  out[b] = softmax(scores + node_mask[b])

x (and W) are cast to fp16 on-chip (SWDGE cast-DMA / DVE copy); matmul
accumulation is f32 in PSUM; A is reduced in f32, Q in fp16.
"""

import os
from contextlib import ExitStack

import numpy as np

import concourse.bass as bass
import concourse.bacc as bacc
import concourse.mybir as mybir
import concourse.tile as tile
from concourse.masks import make_identity
from concourse.tile_rust import add_dep_helper

F32 = mybir.dt.float32
F16 = mybir.dt.float16
AF = mybir.ActivationFunctionType
ALU = mybir.AluOpType

B, N, P, H = 32, 512, 8, 300
NCORES = 8
BL = B // NCORES          # 4 batches per core
NCH = N // 128            # 4 n-chunks of 128
HCB = [0, 128, 256, 300]  # h-chunk boundaries (3 chunks: 128,128,44)
NHC = 3
EPS2 = 1e-24              # matches max(sqrt(Q), 1e-12) == sqrt(Q + 1e-24)


def build_nc():
    nc = bacc.Bacc("TRN2", target_bir_lowering=False)

    x = nc.dram_tensor("x", [BL, N, P, H], F32, kind="ExternalInput")
    instr = nc.dram_tensor("instr", [BL, H], F32, kind="ExternalInput")
    sims = nc.dram_tensor("sims", [BL, P], F32, kind="ExternalInput")
    mask = nc.dram_tensor("mask", [BL, N], F32, kind="ExternalInput")
    Wt = nc.dram_tensor("Wt", [P, H, H], F32, kind="ExternalInput")
    wst = nc.dram_tensor("wst", [H], F32, kind="ExternalInput")
    out = nc.dram_tensor("out", [BL, N], F32, kind="ExternalOutput")

    with tile.TileContext(nc) as tc, ExitStack() as ctx:
        consts = ctx.enter_context(tc.tile_pool(name="consts", bufs=1))
        xraw_p = ctx.enter_context(tc.tile_pool(name="xraw", bufs=5))
        xt_p = ctx.enter_context(tc.tile_pool(name="xt", bufs=24))
        work = ctx.enter_context(tc.tile_pool(name="work", bufs=2))
        acc_p = ctx.enter_context(tc.tile_pool(name="acc", bufs=2))
        psum_t = ctx.enter_context(tc.tile_pool(name="psumt", bufs=2, space="PSUM"))
        psum_z = ctx.enter_context(tc.tile_pool(name="psumz", bufs=3, space="PSUM"))

        # ---------------- constants ----------------
        ident = consts.tile([128, 128], F32)
        make_identity(nc, ident)
        # W in [h, (p, k)] layout (fp16), 3 h-chunk tiles
        w_tiles = []
        w_r = Wt[:].rearrange("p h k -> h p k")
        for hc in range(NHC):
            h0, h1 = HCB[hc], HCB[hc + 1]
            wt_sb = xraw_p.tile([128, P, H], F32, name=f"wt_sb{hc}", tag="wstage", bufs=1)
            nc.sync.dma_start(out=wt_sb[: h1 - h0], in_=w_r[h0:h1])
            w16_sb = consts.tile([128, P, H], mybir.dt.float32r, name=f"w16_sb{hc}")
            nc.vector.tensor_copy(out=w16_sb[: h1 - h0], in_=wt_sb[: h1 - h0])
            w_tiles.append(w16_sb)

        # w_state replicated across 128 partitions
        wst_sb = consts.tile([128, H], F32)
        nc.gpsimd.dma_start(
            out=wst_sb,
            in_=bass.AP(tensor=wst[:].tensor, offset=0, ap=[[0, 128], [1, H]]),
        )
        # sims replicated: [128, BL*P]
        sims_sb = consts.tile([128, BL * P], F32)
        nc.gpsimd.dma_start(
            out=sims_sb,
            in_=bass.AP(tensor=sims[:].tensor, offset=0, ap=[[0, 128], [1, BL * P]]),
        )
        instr_sb = consts.tile([BL, H], F32)
        nc.sync.dma_start(out=instr_sb, in_=instr[:])
        mask_sb = consts.tile([BL, N], F32)
        nc.sync.dma_start(out=mask_sb, in_=mask[:])

        # transpose instr chunks -> instrT[hc] [h_c, BL]
        instrT = []
        for hc in range(NHC):
            h0, h1 = HCB[hc], HCB[hc + 1]
            tpi = psum_t.tile([128, 256], F32, name=f"tp_i{hc}", tag="tp2", bufs=2)
            nc.tensor.transpose(
                out=tpi[: h1 - h0, :BL],
                in_=instr_sb[:, h0:h1],
                identity=ident[:BL, :BL],
            )
            it = consts.tile([128, BL], F32, name=f"instrT{hc}")
            nc.scalar.copy(out=it[: h1 - h0], in_=tpi[: h1 - h0, :BL])
            instrT.append(it)

        # instrsim[hc] [h_c, BL*P] = instr[h,b] * sim[b,p]
        instrsim = []
        for hc in range(NHC):
            h0, h1 = HCB[hc], HCB[hc + 1]
            hsz = h1 - h0
            ism = consts.tile([128, BL * P], F32, name=f"instrsim{hc}")
            for b in range(BL):
                nc.vector.tensor_scalar_mul(
                    out=ism[:hsz, b * P : (b + 1) * P],
                    in0=sims_sb[:hsz, b * P : (b + 1) * P],
                    scalar1=instrT[hc][:hsz, b : b + 1],
                )
            instrsim.append(ism)

        # scores accumulator [128, BL*NCH] (col = b*NCH + nchunk)
        scoresAll = consts.tile([128, BL * NCH], F32)
        eps_sb = consts.tile([128, 1], F32)
        nc.vector.memset(eps_sb, EPS2)

        # ---------------- per-batch pipeline ----------------
        prev_exp = None
        for b in range(BL):
            # load x chunks [128n, P*H] as fp16 (SWDGE cast-DMA)
            xr_tiles = []
            for ncn in range(NCH):
                xr = xraw_p.tile([128, P * H], F32, name=f"xr{b}_{ncn}", tag="xr")
                nc.sync.dma_start(
                    out=xr,
                    in_=x[b, ncn * 128 : (ncn + 1) * 128].rearrange("n p h -> n (p h)"),
                )
                xr_tiles.append(xr)

            # PE transposes (fp16) -> psum, evacuated by DVE copy (2x)
            xt_tiles = {}
            for p in range(P):
                for hc in range(NHC):
                    h0, h1 = HCB[hc], HCB[hc + 1]
                    hsz = h1 - h0
                    tp = psum_t.tile([128, 512], F32, name=f"tp{b}_{p}_{hc}", tag="tp2", bufs=2)
                    for ncn in range(NCH):
                        nc.tensor.transpose(
                            out=tp[:hsz, ncn * 128 : (ncn + 1) * 128],
                            in_=xr_tiles[ncn][:, p * H + h0 : p * H + h1],
                            identity=ident,
                        )
                    xt = xt_p.tile([128, 512], mybir.dt.float32r, name=f"xt{b}_{p}_{hc}", tag="xt")
                    scale = instrsim[hc][:hsz, b * P + p : b * P + p + 1]
                    if (p * NHC + hc) % 2 == 0:
                        nc.vector.tensor_scalar_mul(
                            out=xt[:hsz], in0=tp[:hsz], scalar1=scale
                        )
                    else:
                        nc.scalar.activation(
                            out=xt[:hsz], in_=tp[:hsz], func=AF.Copy, scale=scale
                        )
                    xt_tiles[(p, hc)] = xt

            # ---- phase 1: matmuls, reductions, Ln (table set: natural_log) ----
            units = []
            ln_insts = []
            for ncn in range(NCH):
                aq = []
                qq = []
                for pq in range(P // 2):
                    zt = psum_z.tile(
                        [128, 2, 512], F32, name=f"z{b}_{ncn}_{pq}", tag="z"
                    )
                    for j in range(2):
                        p = pq * 2 + j
                        for hc in range(NHC):
                            h0, h1 = HCB[hc], HCB[hc + 1]
                            hsz = h1 - h0
                            nc.tensor.matmul(
                                zt[:, j, :H],
                                xt_tiles[(p, hc)][:hsz, ncn * 128 : (ncn + 1) * 128],
                                w_tiles[hc][:hsz, p],
                                start=(hc == 0),
                                stop=(hc == NHC - 1),
                            )
                    # squares (ACT; Square is in every relevant set) -> fp16
                    sq = work.tile([128, 2, H], F32, name=f"sq{b}_{ncn}_{pq}", tag="sq")
                    nc.scalar.activation(out=sq, in_=zt[:, :, :H], func=AF.Square)
                    # A pair-sum (DVE reduce over the p-pair, one PSUM operand)
                    a1 = acc_p.tile(
                        [128, H], F32, name=f"a1_{b}_{ncn}_{pq}", tag="a1", bufs=5
                    )
                    nc.vector.tensor_reduce(
                        out=a1,
                        in_=zt[:, :, :H].rearrange("n p k -> n k p"),
                        axis=mybir.AxisListType.X,
                        op=ALU.add,
                    )
                    aq.append(a1)
                    # Q pair-sum (DVE fp16 2x)
                    q1 = acc_p.tile(
                        [128, H], F32, name=f"q1_{b}_{ncn}_{pq}", tag="q1", bufs=5
                    )
                    nc.vector.tensor_add(out=q1, in0=sq[:, 0], in1=sq[:, 1])
                    qq.append(q1)

                # combine pair sums: A on gpsimd (f32), Q mixed
                a01 = acc_p.tile([128, H], F32, name=f"a01_{b}_{ncn}", tag="a01")
                nc.gpsimd.tensor_add(out=a01, in0=aq[0], in1=aq[1])
                a23 = acc_p.tile([128, H], F32, name=f"a23_{b}_{ncn}", tag="a23")
                nc.gpsimd.tensor_add(out=a23, in0=aq[2], in1=aq[3])
                A = work.tile([128, H], F32, name=f"A_{b}_{ncn}", tag="A", bufs=4)
                nc.gpsimd.tensor_add(out=A, in0=a01, in1=a23)

                q01 = acc_p.tile([128, H], F32, name=f"q01_{b}_{ncn}", tag="q01")
                nc.vector.tensor_add(out=q01, in0=qq[0], in1=qq[1])
                q23 = acc_p.tile([128, H], F32, name=f"q23_{b}_{ncn}", tag="q23")
                nc.vector.tensor_add(out=q23, in0=qq[2], in1=qq[3])
                Q = work.tile([128, H], F32, name=f"Q_{b}_{ncn}", tag="Q", bufs=5)
                nc.vector.tensor_add(out=Q, in0=q01, in1=q23)

                # u = ln(Q + eps^2)   (set: natural_log — batched per b)
                u = work.tile([128, H], F32, name=f"u_{b}_{ncn}", tag="u", bufs=4)
                ln_i = nc.scalar.activation(out=u, in_=Q, func=AF.Ln, bias=eps_sb)
                ln_insts.append(ln_i)
                units.append((A, u))

            # ---- phase 2: Exp-based chain (table set: exp) ----
            for ncn in range(NCH):
                A, u = units[ncn]
                r = work.tile([128, H], F32, name=f"r_{b}_{ncn}", tag="r")
                r_i = nc.scalar.activation(out=r, in_=u, func=AF.Exp, scale=-0.5)
                if ncn == 0:
                    for li in ln_insts:
                        add_dep_helper(r_i.ins, li.ins, sync=False,
                                       reason="ACT table phase: exp after all ln")
                if prev_exp is not None and ncn == 0:
                    for li in ln_insts:
                        add_dep_helper(li.ins, prev_exp.ins, sync=False,
                                       reason="ACT table phase: ln after prev-b exp")
                s = work.tile([128, H], F32, name=f"s_{b}_{ncn}", tag="s")
                nc.vector.tensor_mul(out=s, in0=A, in1=r)
                m0 = work.tile([128, H], F32, name=f"m0_{b}_{ncn}", tag="m0")
                nc.scalar.activation(out=m0, in_=s, func=AF.Relu)
                xm = work.tile([128, H], F32, name=f"xm_{b}_{ncn}", tag="xm")
                nc.scalar.activation(out=xm, in_=s, func=AF.Relu, scale=-1.0)
                e = work.tile([128, H], F32, name=f"e_{b}_{ncn}", tag="e")
                e_i = nc.scalar.activation(out=e, in_=xm, func=AF.Exp, scale=-1.0)
                prev_exp = e_i
                s2 = work.tile([128, H], F32, name=f"s2_{b}_{ncn}", tag="s2")
                nc.gpsimd.tensor_add(out=s2, in0=m0, in1=e)
                t = work.tile([128, H], F32, name=f"t_{b}_{ncn}", tag="t")
                col = b * NCH + ncn
                nc.vector.scalar_tensor_tensor(
                    out=t,
                    in0=s2,
                    scalar=1.0,
                    in1=wst_sb,
                    op0=ALU.bypass,
                    op1=ALU.mult,
                    accum_out=scoresAll[:, col : col + 1],
                )

        # ---------------- softmax over n (all 4 b at once) ----------------
        tps = psum_t.tile([128, 256], F32, name="tps", tag="tp2", bufs=2)
        nc.tensor.transpose(out=tps[:16, :128], in_=scoresAll, identity=ident)
        scT = consts.tile([16, 128], F32)
        nc.scalar.copy(out=scT, in_=tps[:16, :128])
        sc4 = consts.tile([BL, N], F32)
        nc.sync.dma_start(out=sc4, in_=scT)
        lg = consts.tile([BL, N], F32)
        nc.vector.tensor_add(out=lg, in0=sc4, in1=mask_sb)
        negmax = consts.tile([BL, 1], F32)
        nc.vector.tensor_reduce(
            out=negmax, in_=lg, axis=mybir.AxisListType.X, op=ALU.max, negate=True
        )
        ex = consts.tile([BL, N], F32)
        esum = consts.tile([BL, 1], F32)
        nc.scalar.activation(out=ex, in_=lg, func=AF.Exp, bias=negmax, accum_out=esum)
        einv = consts.tile([BL, 1], F32)
        nc.vector.reciprocal(out=einv, in_=esum)
        prob = consts.tile([BL, N], F32)
        nc.vector.tensor_scalar_mul(out=prob, in0=ex, scalar1=einv)
        nc.sync.dma_start(out=out[:], in_=prob)

    nc.finalize()
    return nc


_NC_CACHE = {}


def _get_nc():
    if "k" not in _NC_CACHE:
        _NC_CACHE["k"] = build_nc()
    return _NC_CACHE["k"]


def kernel(
    node_attr,
    edge_attr=None,
    instruction=None,
    distribution=None,
    ins_id=None,
    node_prop_similarities=None,
    node_mask=None,
    W_node=None,
    w_state=None,
    **unused,
):
    from concourse.bass_utils import run_bass_kernel_spmd

    node_attr = np.ascontiguousarray(node_attr, dtype=np.float32)
    instruction = np.ascontiguousarray(instruction, dtype=np.float32)
    node_prop_similarities = np.ascontiguousarray(
        node_prop_similarities, dtype=np.float32
    )
    node_mask = np.ascontiguousarray(node_mask, dtype=np.float32)
    W_node = np.ascontiguousarray(W_node, dtype=np.float32)
    w_state = np.ascontiguousarray(w_state, dtype=np.float32)

    nc = _get_nc()
    in_maps = []
    for c in range(NCORES):
        sl = slice(c * BL, (c + 1) * BL)
        in_maps.append(
            {
                "x": node_attr[sl],
                "instr": instruction[sl],
                "sims": node_prop_similarities[sl],
                "mask": node_mask[sl],
                "Wt": W_node,
                "wst": w_state,
            }
        )
    res = run_bass_kernel_spmd(
        nc,
        in_maps,
        core_ids=list(range(NCORES)),
        trace=bool(int(os.environ.get("KERNEL_TRACE", "0"))),
    )
    outs = [r["out"] for r in res.results]
    full = np.concatenate(outs, axis=0)
    if getattr(res, "exec_time_ns", None):
        kernel.last_exec_time_ns = res.exec_time_ns
    kernel.last_result = res
    return full


kernel.last_exec_time_ns = None
kernel.last_result = None


# revision 23
# speedup vs baseline: 1.0420x; 1.0420x over previous
"""NSMCell (ins_id=0 branch) Trainium2 Bass kernel.

Full-input contract: kernel(**inputs) takes the unsharded numpy inputs and
returns the full (32, 512) softmax output. Internally shards batch B=32
across 8 NeuronCores (4 batches each); all compute per (b, n) is local to a
core, so no collectives are needed.

Math (per core, b local):
  z[p,n,k] = sum_h x[b,n,p,h] * instr[b,h] * sim[b,p] * W[p,h,k]
  A = sum_p z ;  Q = sum_p z^2
  s = A * exp(-0.5 * ln(Q + 1e-24))          # == A / max(sqrt(Q), 1e-12)
  e2 = max(s,0) + exp(min(s,0))              # == elu(s) + 1 (softmax-invariant)
  scores[n] = sum_k e2[n,k] * w_state[k]     # + const, softmax-invariant
  out[b] = softmax(scores + node_mask[b])

x (and W) are cast to fp16 on-chip (SWDGE cast-DMA / DVE copy); matmul
accumulation is f32 in PSUM; A is reduced in f32, Q in fp16.
"""

import os
from contextlib import ExitStack

import numpy as np

import concourse.bass as bass
import concourse.bacc as bacc
import concourse.mybir as mybir
import concourse.tile as tile
from concourse.masks import make_identity
from concourse.tile_rust import add_dep_helper

F32 = mybir.dt.float32
F16 = mybir.dt.float16
AF = mybir.ActivationFunctionType
ALU = mybir.AluOpType

B, N, P, H = 32, 512, 8, 300
NCORES = 8
BL = B // NCORES          # 4 batches per core
NCH = N // 128            # 4 n-chunks of 128
HCB = [0, 128, 256, 300]  # h-chunk boundaries (3 chunks: 128,128,44)
NHC = 3
EPS2 = 1e-24              # matches max(sqrt(Q), 1e-12) == sqrt(Q + 1e-24)


def build_nc():
    nc = bacc.Bacc("TRN2", target_bir_lowering=False)

    x = nc.dram_tensor("x", [BL, N, P, H], F32, kind="ExternalInput")
    instr = nc.dram_tensor("instr", [BL, H], F32, kind="ExternalInput")
    sims = nc.dram_tensor("sims", [BL, P], F32, kind="ExternalInput")
    mask = nc.dram_tensor("mask", [BL, N], F32, kind="ExternalInput")
    Wt = nc.dram_tensor("Wt", [P, H, H], F32, kind="ExternalInput")
    wst = nc.dram_tensor("wst", [H], F32, kind="ExternalInput")
    out = nc.dram_tensor("out", [BL, N], F32, kind="ExternalOutput")

    with tile.TileContext(nc) as tc, ExitStack() as ctx:
        consts = ctx.enter_context(tc.tile_pool(name="consts", bufs=1))
        xraw_p = ctx.enter_context(tc.tile_pool(name="xraw", bufs=5))
        xt_p = ctx.enter_context(tc.tile_pool(name="xt", bufs=24))
        work = ctx.enter_context(tc.tile_pool(name="work", bufs=2))
        acc_p = ctx.enter_context(tc.tile_pool(name="acc", bufs=2))
        psum_t = ctx.enter_context(tc.tile_pool(name="psumt", bufs=2, space="PSUM"))
        psum_z = ctx.enter_context(tc.tile_pool(name="psumz", bufs=2, space="PSUM"))

        # ---------------- constants ----------------
        ident = consts.tile([128, 128], F32)
        make_identity(nc, ident)
        # W in [h, (p, k)] layout (fp16), 3 h-chunk tiles
        w_tiles = []
        w_r = Wt[:].rearrange("p h k -> h p k")
        for hc in range(NHC):
            h0, h1 = HCB[hc], HCB[hc + 1]
            wt_sb = xraw_p.tile([128, P, H], F32, name=f"wt_sb{hc}", tag="wstage", bufs=1)
            nc.sync.dma_start(out=wt_sb[: h1 - h0], in_=w_r[h0:h1])
            w16_sb = consts.tile([128, P, H], mybir.dt.float32r, name=f"w16_sb{hc}")
            nc.vector.tensor_copy(out=w16_sb[: h1 - h0], in_=wt_sb[: h1 - h0])
            w_tiles.append(w16_sb)

        # w_state replicated across 128 partitions
        wst_sb = consts.tile([128, H], F32)
        nc.gpsimd.dma_start(
            out=wst_sb,
            in_=bass.AP(tensor=wst[:].tensor, offset=0, ap=[[0, 128], [1, H]]),
        )
        # sims replicated: [128, BL*P]
        sims_sb = consts.tile([128, BL * P], F32)
        nc.gpsimd.dma_start(
            out=sims_sb,
            in_=bass.AP(tensor=sims[:].tensor, offset=0, ap=[[0, 128], [1, BL * P]]),
        )
        instr_sb = consts.tile([BL, H], F32)
        nc.sync.dma_start(out=instr_sb, in_=instr[:])
        mask_sb = consts.tile([BL, N], F32)
        nc.sync.dma_start(out=mask_sb, in_=mask[:])

        # transpose instr chunks -> instrT[hc] [h_c, BL]
        instrT = []
        for hc in range(NHC):
            h0, h1 = HCB[hc], HCB[hc + 1]
            tpi = psum_t.tile([128, 256], F32, name=f"tp_i{hc}", tag="tp2", bufs=4)
            nc.tensor.transpose(
                out=tpi[: h1 - h0, :BL],
                in_=instr_sb[:, h0:h1],
                identity=ident[:BL, :BL],
            )
            it = consts.tile([128, BL], F32, name=f"instrT{hc}")
            nc.scalar.copy(out=it[: h1 - h0], in_=tpi[: h1 - h0, :BL])
            instrT.append(it)

        # instrsim[hc] [h_c, BL*P] = instr[h,b] * sim[b,p]
        instrsim = []
        for hc in range(NHC):
            h0, h1 = HCB[hc], HCB[hc + 1]
            hsz = h1 - h0
            ism = consts.tile([128, BL * P], F32, name=f"instrsim{hc}")
            for b in range(BL):
                nc.vector.tensor_scalar_mul(
                    out=ism[:hsz, b * P : (b + 1) * P],
                    in0=sims_sb[:hsz, b * P : (b + 1) * P],
                    scalar1=instrT[hc][:hsz, b : b + 1],
                )
            instrsim.append(ism)

        # scores accumulator [128, BL*NCH] (col = b*NCH + nchunk)
        scoresAll = consts.tile([128, BL * NCH], F32)
        eps_sb = consts.tile([128, 1], F32)
        nc.vector.memset(eps_sb, EPS2)

        # ---------------- per-batch pipeline ----------------
        prev_exp = None
        for b in range(BL):
            # load x chunks [128n, P*H] as fp16 (SWDGE cast-DMA)
            xr_tiles = []
            for ncn in range(NCH):
                xr = xraw_p.tile([128, P * H], F32, name=f"xr{b}_{ncn}", tag="xr")
                nc.sync.dma_start(
                    out=xr,
                    in_=x[b, ncn * 128 : (ncn + 1) * 128].rearrange("n p h -> n (p h)"),
                )
                xr_tiles.append(xr)

            # PE transposes (fp16) -> psum, evacuated by DVE copy (2x)
            xt_tiles = {}
            for p in range(P):
                for hc in range(NHC):
                    h0, h1 = HCB[hc], HCB[hc + 1]
                    hsz = h1 - h0
                    tp = psum_t.tile([128, 512], F32, name=f"tp{b}_{p}_{hc}", tag="tp2", bufs=4)
                    for ncn in range(NCH):
                        nc.tensor.transpose(
                            out=tp[:hsz, ncn * 128 : (ncn + 1) * 128],
                            in_=xr_tiles[ncn][:, p * H + h0 : p * H + h1],
                            identity=ident,
                        )
                    xt = xt_p.tile([128, 512], mybir.dt.float32r, name=f"xt{b}_{p}_{hc}", tag="xt")
                    scale = instrsim[hc][:hsz, b * P + p : b * P + p + 1]
                    if (p * NHC + hc) % 2 == 0:
                        nc.vector.tensor_scalar_mul(
                            out=xt[:hsz], in0=tp[:hsz], scalar1=scale
                        )
                    else:
                        nc.scalar.activation(
                            out=xt[:hsz], in_=tp[:hsz], func=AF.Copy, scale=scale
                        )
                    xt_tiles[(p, hc)] = xt

            # ---- phase 1: matmuls, reductions, Ln (table set: natural_log) ----
            units = []
            ln_insts = []
            for ncn in range(NCH):
                aq = []
                qq = []
                for pq in range(P // 2):
                    zt = psum_z.tile(
                        [128, 2, 512], F32, name=f"z{b}_{ncn}_{pq}", tag="z"
                    )
                    for j in range(2):
                        p = pq * 2 + j
                        for hc in range(NHC):
                            h0, h1 = HCB[hc], HCB[hc + 1]
                            hsz = h1 - h0
                            nc.tensor.matmul(
                                zt[:, j, :H],
                                xt_tiles[(p, hc)][:hsz, ncn * 128 : (ncn + 1) * 128],
                                w_tiles[hc][:hsz, p],
                                start=(hc == 0),
                                stop=(hc == NHC - 1),
                            )
                    # squares (ACT; Square is in every relevant set) -> fp16
                    sq = work.tile([128, 2, H], F32, name=f"sq{b}_{ncn}_{pq}", tag="sq")
                    nc.scalar.activation(out=sq, in_=zt[:, :, :H], func=AF.Square)
                    # A pair-sum (DVE reduce over the p-pair, one PSUM operand)
                    a1 = acc_p.tile(
                        [128, H], F32, name=f"a1_{b}_{ncn}_{pq}", tag="a1", bufs=5
                    )
                    nc.vector.tensor_reduce(
                        out=a1,
                        in_=zt[:, :, :H].rearrange("n p k -> n k p"),
                        axis=mybir.AxisListType.X,
                        op=ALU.add,
                    )
                    aq.append(a1)
                    # Q pair-sum (DVE fp16 2x)
                    q1 = acc_p.tile(
                        [128, H], F32, name=f"q1_{b}_{ncn}_{pq}", tag="q1", bufs=5
                    )
                    nc.vector.tensor_add(out=q1, in0=sq[:, 0], in1=sq[:, 1])
                    qq.append(q1)

                # combine pair sums: A on gpsimd (f32), Q mixed
                a01 = acc_p.tile([128, H], F32, name=f"a01_{b}_{ncn}", tag="a01")
                nc.gpsimd.tensor_add(out=a01, in0=aq[0], in1=aq[1])
                a23 = acc_p.tile([128, H], F32, name=f"a23_{b}_{ncn}", tag="a23")
                nc.gpsimd.tensor_add(out=a23, in0=aq[2], in1=aq[3])
                A = work.tile([128, H], F32, name=f"A_{b}_{ncn}", tag="A", bufs=4)
                nc.gpsimd.tensor_add(out=A, in0=a01, in1=a23)

                q01 = acc_p.tile([128, H], F32, name=f"q01_{b}_{ncn}", tag="q01")
                nc.vector.tensor_add(out=q01, in0=qq[0], in1=qq[1])
                q23 = acc_p.tile([128, H], F32, name=f"q23_{b}_{ncn}", tag="q23")
                nc.vector.tensor_add(out=q23, in0=qq[2], in1=qq[3])
                Q = work.tile([128, H], F32, name=f"Q_{b}_{ncn}", tag="Q", bufs=5)
                nc.vector.tensor_add(out=Q, in0=q01, in1=q23)

                # u = ln(Q + eps^2)   (set: natural_log — batched per b)
                u = work.tile([128, H], F32, name=f"u_{b}_{ncn}", tag="u", bufs=4)
                ln_i = nc.scalar.activation(out=u, in_=Q, func=AF.Ln, bias=eps_sb)
                ln_insts.append(ln_i)
                units.append((A, u))

            # ---- phase 2: Exp-based chain (table set: exp) ----
            for ncn in range(NCH):
                A, u = units[ncn]
                r = work.tile([128, H], F32, name=f"r_{b}_{ncn}", tag="r")
                r_i = nc.scalar.activation(out=r, in_=u, func=AF.Exp, scale=-0.5)
                if ncn == 0:
                    for li in ln_insts:
                        add_dep_helper(r_i.ins, li.ins, sync=False,
                                       reason="ACT table phase: exp after all ln")
                if prev_exp is not None and ncn == 0:
                    for li in ln_insts:
                        add_dep_helper(li.ins, prev_exp.ins, sync=False,
                                       reason="ACT table phase: ln after prev-b exp")
                s = work.tile([128, H], F32, name=f"s_{b}_{ncn}", tag="s")
                nc.vector.tensor_mul(out=s, in0=A, in1=r)
                m0 = work.tile([128, H], F32, name=f"m0_{b}_{ncn}", tag="m0")
                nc.scalar.activation(out=m0, in_=s, func=AF.Relu)
                xm = work.tile([128, H], F32, name=f"xm_{b}_{ncn}", tag="xm")
                nc.scalar.activation(out=xm, in_=s, func=AF.Relu, scale=-1.0)
                e = work.tile([128, H], F32, name=f"e_{b}_{ncn}", tag="e")
                e_i = nc.scalar.activation(out=e, in_=xm, func=AF.Exp, scale=-1.0)
                prev_exp = e_i
                s2 = work.tile([128, H], F32, name=f"s2_{b}_{ncn}", tag="s2")
                nc.gpsimd.tensor_add(out=s2, in0=m0, in1=e)
                t = work.tile([128, H], F32, name=f"t_{b}_{ncn}", tag="t")
                col = b * NCH + ncn
                nc.vector.scalar_tensor_tensor(
                    out=t,
                    in0=s2,
                    scalar=1.0,
                    in1=wst_sb,
                    op0=ALU.bypass,
                    op1=ALU.mult,
                    accum_out=scoresAll[:, col : col + 1],
                )

        # ---------------- softmax over n (all 4 b at once) ----------------
        tps = psum_t.tile([128, 256], F32, name="tps", tag="tp2", bufs=4)
        nc.tensor.transpose(out=tps[:16, :128], in_=scoresAll, identity=ident)
        scT = consts.tile([16, 128], F32)
        nc.scalar.copy(out=scT, in_=tps[:16, :128])
        sc4 = consts.tile([BL, N], F32)
        nc.sync.dma_start(out=sc4, in_=scT)
        lg = consts.tile([BL, N], F32)
        nc.vector.tensor_add(out=lg, in0=sc4, in1=mask_sb)
        negmax = consts.tile([BL, 1], F32)
        nc.vector.tensor_reduce(
            out=negmax, in_=lg, axis=mybir.AxisListType.X, op=ALU.max, negate=True
        )
        ex = consts.tile([BL, N], F32)
        esum = consts.tile([BL, 1], F32)
        nc.scalar.activation(out=ex, in_=lg, func=AF.Exp, bias=negmax, accum_out=esum)
        einv = consts.tile([BL, 1], F32)
        nc.vector.reciprocal(out=einv, in_=esum)
        prob = consts.tile([BL, N], F32)
        nc.vector.tensor_scalar_mul(out=prob, in0=ex, scalar1=einv)
        nc.sync.dma_start(out=out[:], in_=prob)

    nc.finalize()
    return nc


_NC_CACHE = {}


def _get_nc():
    if "k" not in _NC_CACHE:
        _NC_CACHE["k"] = build_nc()
    return _NC_CACHE["k"]


def kernel(
    node_attr,
    edge_attr=None,
    instruction=None,
    distribution=None,
    ins_id=None,
    node_prop_similarities=None,
    node_mask=None,
    W_node=None,
    w_state=None,
    **unused,
):
    from concourse.bass_utils import run_bass_kernel_spmd

    node_attr = np.ascontiguousarray(node_attr, dtype=np.float32)
    instruction = np.ascontiguousarray(instruction, dtype=np.float32)
    node_prop_similarities = np.ascontiguousarray(
        node_prop_similarities, dtype=np.float32
    )
    node_mask = np.ascontiguousarray(node_mask, dtype=np.float32)
    W_node = np.ascontiguousarray(W_node, dtype=np.float32)
    w_state = np.ascontiguousarray(w_state, dtype=np.float32)

    nc = _get_nc()
    in_maps = []
    for c in range(NCORES):
        sl = slice(c * BL, (c + 1) * BL)
        in_maps.append(
            {
                "x": node_attr[sl],
                "instr": instruction[sl],
                "sims": node_prop_similarities[sl],
                "mask": node_mask[sl],
                "Wt": W_node,
                "wst": w_state,
            }
        )
    res = run_bass_kernel_spmd(
        nc,
        in_maps,
        core_ids=list(range(NCORES)),
        trace=bool(int(os.environ.get("KERNEL_TRACE", "0"))),
    )
    outs = [r["out"] for r in res.results]
    full = np.concatenate(outs, axis=0)
    if getattr(res, "exec_time_ns", None):
        kernel.last_exec_time_ns = res.exec_time_ns
    kernel.last_result = res
    return full


kernel.last_exec_time_ns = None
kernel.last_result = None
